# revision 1
# baseline (speedup 1.0000x reference)
"""Trainium2 Bass kernel for BinaryConv2dBBCU_Down.

Pipeline (per image):
  AvgPool2d(2,2) -> +bias -> sign -> 3x3 binary conv (weights scale*sign(w))
  -> +b0 -> PReLU(alpha) -> +b1

Sharding: pure data parallelism, one image per NeuronCore (batch 8 over 8
cores); conv weights / biases / alphas replicated.

Device math:
  a   = Sign(0.25 * (4-elem pool sum) + move0_bias)       (fp8e4, exactly +-1)
  s   = sum over 9 taps of sign(w)^T @ a_shifted + k      (exact in fp32 PSUM)
  out = c1*s + |sA*s_conv + c3*b0|                        (fp32)
where per-output-channel constants (computed on host, fp32):
  scale = mean|w|, c1 = 0.5(1+alpha)*scale, c2 = 0.5(1+alpha)*b0 + b1,
  c3 = 0.5(1-alpha), sA = c3*scale, k ~= c2/c1 (bf16 bias-tap matmul; the
  Abs bias is compensated with the exact rounded k so only the tiny
  c1*(c2/c1 - k) residual remains)
which equals PReLU(scale*s_conv + b0) + b1 for alpha <= 1.

Structure: the image is processed in 8 bands of 16 output rows. Pooling is a
single pass over 16 chunks of 8 pooled rows; each chunk's sign output is
written into the owning band's flat padded tile, and boundary rows are
duplicated into the neighbouring band tile so no x row is ever re-read.
The conv uses fp8 DoubleRow matmuls (two taps per instruction) over the flat
padded layout: each output tile is 2 padded rows (N=260 contiguous), four
such blocks live in the four banks of one PSUM tile, and a whole half-band
drains with a single Abs + scalar_tensor_tensor + DMA.
"""

import sys

sys.path.insert(0, "/opt/trn_rl_repo")

import numpy as np

B, CIN, COUT, H, W = 8, 128, 256, 256, 256
H2, W2 = H // 2, W // 2  # pooled spatial dims (128, 128)
N_CORES = 8
N_BANDS = 8
BAND = H2 // N_BANDS      # 16 output rows per band
N_CHUNKS = 16
CH = H2 // N_CHUNKS       # 8 pooled rows per chunk

_PROGRAMS: dict = {}


def _build_program(repeats: int = 1):
    import concourse.bacc as bacc
    import concourse.tile as tile
    from concourse import mybir

    import concourse.bass as bass_mod
    f32 = mybir.dt.float32
    fp8 = mybir.dt.float8e4
    Act = mybir.ActivationFunctionType
    Alu = mybir.AluOpType
    DoubleRow = mybir.MatmulPerfMode.DoubleRow
    WP = W2 + 2          # padded row length (130)
    FLAT = (BAND + 2) * WP + 2   # flat apad tile size (+1 guard each end)

    nc = bacc.Bacc("TRN2", target_bir_lowering=False, debug=False,
                   num_devices=N_CORES)
    x_in = nc.declare_dram_parameter("x", [CIN, H, W], f32, isOutput=False)
    wt_in = nc.declare_dram_parameter("wt", [CIN, 9, COUT], fp8, isOutput=False)
    ct_in = nc.declare_dram_parameter("ct", [128, 9], f32, isOutput=False)
    kb_in = nc.declare_dram_parameter("kb", [128, COUT], mybir.dt.bfloat16,
                                      isOutput=False)
    y_out = nc.declare_dram_parameter("y", [COUT, H2, W2], f32, isOutput=True)

    with tile.TileContext(nc) as tc:
        with (
            tc.tile_pool(name="consts", bufs=1) as consts,
            tc.tile_pool(name="xch", bufs=4) as xch_pool,
            tc.tile_pool(name="rs", bufs=4) as rs_pool,
            tc.tile_pool(name="cs", bufs=4) as cs_pool,
            tc.tile_pool(name="apad", bufs=4) as apad_pool,
            tc.tile_pool(name="psum", bufs=2, space="PSUM") as psum_pool,
            tc.tile_pool(name="u", bufs=6) as u_pool,
            tc.tile_pool(name="v", bufs=6) as v_pool,
        ):
            wt_sb = consts.tile([CIN, 9, COUT], fp8)
            nc.sync.dma_start(out=wt_sb[:], in_=wt_in[:])
            ct_sb = consts.tile([128, 9], f32)
            nc.sync.dma_start(out=ct_sb[:], in_=ct_in[:])
            kb_sb = consts.tile([128, COUT], mybir.dt.bfloat16)
            nc.sync.dma_start(out=kb_sb[:], in_=kb_in[:])
            ones_sb = consts.tile([128, 512], mybir.dt.bfloat16)
            nc.vector.memset(ones_sb, 1.0)

            for _rep in range(repeats):
                # Padded sign-activation band tiles: band b local row l holds
                # global pooled row 16b-1+l; col p holds global col p-1.
                apad: dict = {}

                def new_band(b):
                    # flat padded band: element (row, col) at 1 + row*WP + col
                    # with one guard element at each end (read by the wrapped
                    # conv windows of the garbage border columns)
                    t = apad_pool.tile([CIN, FLAT], fp8,
                                       name=f"apad{b}", tag="apad")
                    apad[b] = t
                    vw = t[:, 1:1 + (BAND + 2) * WP].rearrange(
                        "p (r c) -> p r c", c=WP)
                    nc.vector.memset(t[:, 0:1], 0.0)
                    nc.vector.memset(t[:, FLAT - 1:FLAT], 0.0)
                    nc.vector.memset(vw[:, :, 0:1], 0.0)
                    nc.vector.memset(vw[:, :, W2 + 1:W2 + 2], 0.0)
                    if b == 0:
                        nc.vector.memset(vw[:, 0:1, :], 0.0)
                    if b == N_BANDS - 1:
                        nc.vector.memset(vw[:, BAND + 1:BAND + 2, :], 0.0)
                    return t

                def band_view(b):
                    t = apad[b]
                    return t[:, 1:1 + (BAND + 2) * WP].rearrange(
                        "p (r c) -> p r c", c=WP)

                def emit_chunk(c):
                    # pooled rows 8c .. 8c+7
                    bm = c // 2
                    if bm not in apad:
                        new_band(bm)
                    xt = xch_pool.tile([CIN, 2 * CH, W], f32)
                    nc.sync.dma_start(out=xt,
                                      in_=x_in[:, 2 * CH * c:2 * CH * (c + 1), :])
                    xv = xt.rearrange("p (r two) w -> p r two w", two=2)
                    rt = rs_pool.tile([CIN, CH, W], f32)
                    nc.vector.tensor_add(out=rt, in0=xv[:, :, 0, :],
                                         in1=xv[:, :, 1, :])
                    rv = rt.rearrange("p r (w two) -> p r w two", two=2)
                    cst = cs_pool.tile([CIN, CH, W2], f32)
                    # column-pair sum on GpSimd; DVE keeps only the row sum
                    nc.gpsimd.tensor_add(out=cst, in0=rv[:, :, :, 0],
                                         in1=rv[:, :, :, 1])
                    # main write: even chunk -> local rows 1..8,
                    # odd chunk -> local rows 9..16
                    l = 1 + CH * (c - 2 * bm)
                    nc.scalar.activation(out=band_view(bm)[:, l:l + CH, 1:W2 + 1],
                                         in_=cst, func=Act.Sign,
                                         bias=ct_sb[:, 0:1], scale=0.25)
                    if c % 2 == 0 and bm > 0:
                        # first row is also band bm-1's bottom halo (row 17)
                        nc.scalar.activation(
                            out=band_view(bm - 1)[:, BAND + 1:BAND + 2, 1:W2 + 1],
                            in_=cst[:, 0:1, :], func=Act.Sign,
                            bias=ct_sb[:, 0:1], scale=0.25)
                    if c % 2 == 1 and bm < N_BANDS - 1:
                        # last row is also band bm+1's top halo (row 0)
                        if bm + 1 not in apad:
                            new_band(bm + 1)
                        nc.scalar.activation(
                            out=band_view(bm + 1)[:, 0:1, 1:W2 + 1],
                            in_=cst[:, CH - 1:CH, :], func=Act.Sign,
                            bias=ct_sb[:, 0:1], scale=0.25)

                # Each half-band (8 output rows) is computed per channel
                # half as four uniform 2-row blocks (N = 2*WP = 260) living
                # in the four banks of ONE PSUM tile, so the whole half-band
                # drains with a single Abs + scalar_tensor_tensor + DMA on a
                # [128, 4, 260] access pattern. Cols 0 and WP-1 of each row
                # are garbage lanes the output DMA skips.
                NB = 2 * WP  # 260

                def emit_conv(b, half):
                    ap_t = apad[b]
                    for h in (0, 1):
                        c0 = 1 + 4 * h
                        c1_ap = ct_sb[:, c0:c0 + 1]
                        sA_ap = ct_sb[:, c0 + 2:c0 + 3]
                        bA_ap = ct_sb[:, c0 + 3:c0 + 4]
                        pt4 = psum_pool.tile([128, 4, 512], f32,
                                             name="pt4", tag="pt4")
                        outs = [pt4[:, k, 0:NB] for k in range(4)]
                        rbase = [8 * half + 2 * k for k in range(4)]
                        # bf16 bias tap (K=128, lhsT rows all k/128) seeds
                        # each PSUM block with k = c2/c1 so the epilogue is a
                        # single scalar_tensor_tensor
                        for po in outs:
                            nc.tensor.matmul(
                                po, kb_sb[:, h * 128:(h + 1) * 128],
                                ones_sb[:, :NB],
                                start=True, stop=False)
                        # fp8 DoubleRow: tap pairs (0,1)(2,3)(4,5)(6,7) run
                        # two K=128 contractions per instruction; tap 8 is a
                        # plain fp8 matmul. tap-major keeps lhsT stationary.
                        for t in (0, 2, 4, 6, 8):
                            ky, kx = divmod(t, 3)
                            dt0 = (ky - 1) * WP + (kx - 1)
                            if t < 8:
                                ky2, kx2 = divmod(t + 1, 3)
                                dpair = (ky2 - ky) * WP + (kx2 - kx)
                                lhs = wt_sb[:, t:t + 2, h * 128:(h + 1) * 128]
                            else:
                                lhs = wt_sb[:, t, h * 128:(h + 1) * 128]
                            for r, po in zip(rbase, outs):
                                base = 1 + (r + 1) * WP + dt0
                                r0 = ap_t[:, base:base + NB]
                                if t < 8:
                                    rhs = bass_mod.AP(
                                        tensor=r0.tensor, offset=r0.offset,
                                        ap=[r0.ap[0], [dpair, 2], r0.ap[1]])
                                    nc.tensor.matmul(po, lhs, rhs,
                                                     start=False,
                                                     stop=False,
                                                     perf_mode=DoubleRow)
                                else:
                                    nc.tensor.matmul(po, lhs, r0,
                                                     start=False, stop=True)
                        pv = pt4[:, :, 0:NB]
                        ut = u_pool.tile([128, 4, NB], f32, name="ut",
                                         tag="ut")
                        nc.scalar.activation(out=ut, in_=pv, func=Act.Abs,
                                             bias=bA_ap, scale=sA_ap)
                        vt = v_pool.tile([128, 4, NB], f32, name="vt",
                                         tag="vt")
                        # out = c1*(s+k) + |sA*(s+k) + bA| in one DVE op
                        nc.vector.scalar_tensor_tensor(
                            out=vt, in0=pv, scalar=c1_ap, in1=ut,
                            op0=Alu.mult, op1=Alu.add)
                        # output DMA on the Activation HWDGE: cross-engine
                        # queue mixing costs bandwidth when both streams are
                        # saturated, but at the real pacing (one 4KB write
                        # per ~1.5us) it measures faster than sharing the SP
                        # queue, where a result that isn't ready yet blocks
                        # queued input DMAs (measured: ACT ~160us vs SP
                        # ~181us end-to-end)
                        y0 = BAND * b + 8 * half
                        vv = vt.rearrange("p f (r c) -> p f r c", c=WP)
                        nc.scalar.dma_start(
                            out=y_out[h * 128:(h + 1) * 128, y0:y0 + 8, :],
                            in_=vv[:, :, :, 1:W2 + 1])

                # half-band granularity: the first half of band b only
                # needs pooled rows up to 16b+8 (chunk 2b+1), the second
                # half needs chunk 2b+2's halo row
                for c in range(N_CHUNKS):
                    emit_chunk(c)
                    if c % 2 == 1:
                        emit_conv(c // 2, 0)
                    elif c >= 2:
                        emit_conv(c // 2 - 1, 1)
                        apad.pop(c // 2 - 1)
                emit_conv(N_BANDS - 1, 1)
                apad.pop(N_BANDS - 1)
    nc.compile()
    return nc


def get_program(repeats: int = 1):
    if repeats not in _PROGRAMS:
        _PROGRAMS[repeats] = _build_program(repeats)
    return _PROGRAMS[repeats]


def host_prep(weight, move0_bias, pr_bias0, prelu_alpha, pr_bias1):
    import ml_dtypes

    w = np.asarray(weight, dtype=np.float32)  # [COUT, CIN, 3, 3]
    sw = np.sign(w).astype(np.float32)
    # lhsT layout [ci, tap, co]
    wt = np.ascontiguousarray(
        np.transpose(sw, (1, 2, 3, 0)).reshape(CIN, 9, COUT)
    ).astype(ml_dtypes.float8_e4m3)

    scale = np.mean(np.abs(w), axis=(1, 2, 3), dtype=np.float32)  # [COUT]
    al = np.asarray(prelu_alpha, dtype=np.float32).reshape(COUT)
    b0 = np.asarray(pr_bias0, dtype=np.float32).reshape(COUT)
    b1 = np.asarray(pr_bias1, dtype=np.float32).reshape(COUT)
    c1 = 0.5 * (1.0 + al) * scale
    c2 = 0.5 * (1.0 + al) * b0 + b1
    c3 = 0.5 * (1.0 - al)
    sA = c3 * scale
    bA = c3 * b0

    # bias tap: 128 lhsT rows of bf16(k/128) summed by a ones matmul.
    # Compensate the Abs bias with the exact summed value so only the tiny
    # c1*(k - k_eff) residual remains.
    kq = (c2 / c1 / 128.0).astype(ml_dtypes.bfloat16)
    k_eff = 128.0 * kq.astype(np.float32)
    bA = bA - sA * k_eff
    kb = np.broadcast_to(kq.reshape(1, COUT), (128, COUT)).copy()

    ct = np.zeros((128, 9), dtype=np.float32)
    ct[:, 0] = np.asarray(move0_bias, dtype=np.float32).reshape(CIN)
    for h in (0, 1):
        sl = slice(h * 128, (h + 1) * 128)
        ct[:, 1 + 4 * h] = c1[sl]
        ct[:, 2 + 4 * h] = c2[sl]
        ct[:, 3 + 4 * h] = sA[sl]
        ct[:, 4 + 4 * h] = bA[sl]
    return wt, ct, kb


def kernel(x, weight, move0_bias, pr_bias0, prelu_alpha, pr_bias1):
    from concourse.bass_utils import run_bass_kernel_spmd

    x = np.asarray(x, dtype=np.float32)
    wt, ct, kb = host_prep(weight, move0_bias, pr_bias0, prelu_alpha,
                           pr_bias1)
    nc = get_program()
    in_maps = [{"x": x[c], "wt": wt, "ct": ct, "kb": kb}
               for c in range(N_CORES)]
    res = run_bass_kernel_spmd(nc, in_maps, list(range(N_CORES)))
    y = np.stack([res.results[c]["y"] for c in range(N_CORES)], axis=0)
    return np.ascontiguousarray(y.astype(np.float32))



# revision 4
# speedup vs baseline: 1.2565x; 1.2565x over previous
"""Trainium2 Bass kernel for BinaryConv2dBBCU_Down.

Pipeline (per image):
  AvgPool2d(2,2) -> +bias -> sign -> 3x3 binary conv (weights scale*sign(w))
  -> +b0 -> PReLU(alpha) -> +b1

Sharding: pure data parallelism, one image per NeuronCore (batch 8 over 8
cores); conv weights / biases / alphas replicated.

Device math:
  a   = Sign(0.25 * (4-elem pool sum) + move0_bias)       (fp8e4, exactly +-1)
  s   = sum over 9 taps of sign(w)^T @ a_shifted + k      (exact in fp32 PSUM)
  out = c1*s + |sA*s_conv + c3*b0|                        (fp32)
where per-output-channel constants (computed on host, fp32):
  scale = mean|w|, c1 = 0.5(1+alpha)*scale, c2 = 0.5(1+alpha)*b0 + b1,
  c3 = 0.5(1-alpha), sA = c3*scale, k ~= c2/c1 (bf16 bias-tap matmul; the
  Abs bias is compensated with the exact rounded k so only the tiny
  c1*(c2/c1 - k) residual remains)
which equals PReLU(scale*s_conv + b0) + b1 for alpha <= 1.

Structure: the image is processed in 8 bands of 16 output rows. Pooling is a
single pass over 16 chunks of 8 pooled rows; each chunk's sign output is
written into the owning band's flat padded tile, and boundary rows are
duplicated into the neighbouring band tile so no x row is ever re-read.
The conv uses fp8 DoubleRow matmuls (two taps per instruction) over the flat
padded layout: each output tile is 2 padded rows (N=260 contiguous), four
such blocks live in the four banks of one PSUM tile, and a whole half-band
drains with a single Abs + scalar_tensor_tensor + DMA.
"""

import sys

sys.path.insert(0, "/opt/trn_rl_repo")

import numpy as np

B, CIN, COUT, H, W = 8, 128, 256, 256, 256
H2, W2 = H // 2, W // 2  # pooled spatial dims (128, 128)
N_CORES = 8
N_BANDS = 8
BAND = H2 // N_BANDS      # 16 output rows per band
N_CHUNKS = 16
CH = H2 // N_CHUNKS       # 8 pooled rows per chunk

_PROGRAMS: dict = {}


def _build_program(repeats: int = 1):
    import concourse.bacc as bacc
    import concourse.tile as tile
    from concourse import mybir

    import concourse.bass as bass_mod
    f32 = mybir.dt.float32
    fp8 = mybir.dt.float8e4
    Act = mybir.ActivationFunctionType
    Alu = mybir.AluOpType
    DoubleRow = mybir.MatmulPerfMode.DoubleRow
    WP = W2 + 2          # padded row length (130)
    FLAT = (BAND + 2) * WP + 2   # flat apad tile size (+1 guard each end)

    nc = bacc.Bacc("TRN2", target_bir_lowering=False, debug=False,
                   num_devices=N_CORES)
    x_in = nc.declare_dram_parameter("x", [CIN, H, W], f32, isOutput=False)
    wt_in = nc.declare_dram_parameter("wt", [CIN, 9, COUT], fp8, isOutput=False)
    ct_in = nc.declare_dram_parameter("ct", [128, 9], f32, isOutput=False)
    kb_in = nc.declare_dram_parameter("kb", [128, COUT], mybir.dt.bfloat16,
                                      isOutput=False)
    # bf16 output in a padded 130-col layout: keeps every output DMA one
    # contiguous 2080B run per partition (no 512B descriptor fragmentation)
    # and halves write traffic; host strips cols 0/129 and upcasts.
    y_out = nc.declare_dram_parameter("y", [COUT, H2 * WP], mybir.dt.bfloat16,
                                      isOutput=True)

    with tile.TileContext(nc) as tc:
        with (
            tc.tile_pool(name="consts", bufs=1) as consts,
            tc.tile_pool(name="xch", bufs=4) as xch_pool,
            tc.tile_pool(name="rs", bufs=4) as rs_pool,
            tc.tile_pool(name="cs", bufs=4) as cs_pool,
            tc.tile_pool(name="apad", bufs=4) as apad_pool,
            tc.tile_pool(name="psum", bufs=2, space="PSUM") as psum_pool,
            tc.tile_pool(name="u", bufs=6) as u_pool,
            tc.tile_pool(name="v", bufs=6) as v_pool,
        ):
            wt_sb = consts.tile([CIN, 9, COUT], fp8)
            nc.sync.dma_start(out=wt_sb[:], in_=wt_in[:])
            ct_sb = consts.tile([128, 9], f32)
            nc.sync.dma_start(out=ct_sb[:], in_=ct_in[:])
            kb_sb = consts.tile([128, COUT], mybir.dt.bfloat16)
            nc.sync.dma_start(out=kb_sb[:], in_=kb_in[:])
            ones_sb = consts.tile([128, 512], mybir.dt.bfloat16)
            nc.vector.memset(ones_sb, 1.0)

            for _rep in range(repeats):
                # Padded sign-activation band tiles: band b local row l holds
                # global pooled row 16b-1+l; col p holds global col p-1.
                apad: dict = {}

                def new_band(b):
                    # flat padded band: element (row, col) at 1 + row*WP + col
                    # with one guard element at each end (read by the wrapped
                    # conv windows of the garbage border columns)
                    t = apad_pool.tile([CIN, FLAT], fp8,
                                       name=f"apad{b}", tag="apad")
                    apad[b] = t
                    vw = t[:, 1:1 + (BAND + 2) * WP].rearrange(
                        "p (r c) -> p r c", c=WP)
                    nc.vector.memset(t[:, 0:1], 0.0)
                    nc.vector.memset(t[:, FLAT - 1:FLAT], 0.0)
                    nc.vector.memset(vw[:, :, 0:1], 0.0)
                    nc.vector.memset(vw[:, :, W2 + 1:W2 + 2], 0.0)
                    if b == 0:
                        nc.vector.memset(vw[:, 0:1, :], 0.0)
                    if b == N_BANDS - 1:
                        nc.vector.memset(vw[:, BAND + 1:BAND + 2, :], 0.0)
                    return t

                def band_view(b):
                    t = apad[b]
                    return t[:, 1:1 + (BAND + 2) * WP].rearrange(
                        "p (r c) -> p r c", c=WP)

                def emit_chunk(c):
                    # pooled rows 8c .. 8c+7
                    bm = c // 2
                    if bm not in apad:
                        new_band(bm)
                    xt = xch_pool.tile([CIN, 2 * CH, W], f32)
                    nc.sync.dma_start(out=xt,
                                      in_=x_in[:, 2 * CH * c:2 * CH * (c + 1), :])
                    xv = xt.rearrange("p (r two) w -> p r two w", two=2)
                    rt = rs_pool.tile([CIN, CH, W], f32)
                    nc.vector.tensor_add(out=rt, in0=xv[:, :, 0, :],
                                         in1=xv[:, :, 1, :])
                    rv = rt.rearrange("p r (w two) -> p r w two", two=2)
                    cst = cs_pool.tile([CIN, CH, W2], f32)
                    # column-pair sum on GpSimd; DVE keeps only the row sum
                    nc.gpsimd.tensor_add(out=cst, in0=rv[:, :, :, 0],
                                         in1=rv[:, :, :, 1])
                    # main write: even chunk -> local rows 1..8,
                    # odd chunk -> local rows 9..16
                    l = 1 + CH * (c - 2 * bm)
                    nc.scalar.activation(out=band_view(bm)[:, l:l + CH, 1:W2 + 1],
                                         in_=cst, func=Act.Sign,
                                         bias=ct_sb[:, 0:1], scale=0.25)
                    if c % 2 == 0 and bm > 0:
                        # first row is also band bm-1's bottom halo (row 17)
                        nc.scalar.activation(
                            out=band_view(bm - 1)[:, BAND + 1:BAND + 2, 1:W2 + 1],
                            in_=cst[:, 0:1, :], func=Act.Sign,
                            bias=ct_sb[:, 0:1], scale=0.25)
                    if c % 2 == 1 and bm < N_BANDS - 1:
                        # last row is also band bm+1's top halo (row 0)
                        if bm + 1 not in apad:
                            new_band(bm + 1)
                        nc.scalar.activation(
                            out=band_view(bm + 1)[:, 0:1, 1:W2 + 1],
                            in_=cst[:, CH - 1:CH, :], func=Act.Sign,
                            bias=ct_sb[:, 0:1], scale=0.25)

                # Each half-band (8 output rows) is computed per channel
                # half as four uniform 2-row blocks (N = 2*WP = 260) living
                # in the four banks of ONE PSUM tile, so the whole half-band
                # drains with a single Abs + scalar_tensor_tensor + DMA on a
                # [128, 4, 260] access pattern. Cols 0 and WP-1 of each row
                # are garbage lanes the output DMA skips.
                NB = 2 * WP  # 260

                def emit_conv(b, half):
                    ap_t = apad[b]
                    for h in (0, 1):
                        c0 = 1 + 4 * h
                        c1_ap = ct_sb[:, c0:c0 + 1]
                        sA_ap = ct_sb[:, c0 + 2:c0 + 3]
                        bA_ap = ct_sb[:, c0 + 3:c0 + 4]
                        pt4 = psum_pool.tile([128, 4, 512], f32,
                                             name="pt4", tag="pt4")
                        outs = [pt4[:, k, 0:NB] for k in range(4)]
                        rbase = [8 * half + 2 * k for k in range(4)]
                        # bf16 bias tap (K=128, lhsT rows all k/128) seeds
                        # each PSUM block with k = c2/c1 so the epilogue is a
                        # single scalar_tensor_tensor
                        for po in outs:
                            nc.tensor.matmul(
                                po, kb_sb[:, h * 128:(h + 1) * 128],
                                ones_sb[:, :NB],
                                start=True, stop=False)
                        # fp8 DoubleRow: tap pairs (0,1)(2,3)(4,5)(6,7) run
                        # two K=128 contractions per instruction; tap 8 is a
                        # plain fp8 matmul. tap-major keeps lhsT stationary.
                        for t in (0, 2, 4, 6, 8):
                            ky, kx = divmod(t, 3)
                            dt0 = (ky - 1) * WP + (kx - 1)
                            if t < 8:
                                ky2, kx2 = divmod(t + 1, 3)
                                dpair = (ky2 - ky) * WP + (kx2 - kx)
                                lhs = wt_sb[:, t:t + 2, h * 128:(h + 1) * 128]
                            else:
                                lhs = wt_sb[:, t, h * 128:(h + 1) * 128]
                            for r, po in zip(rbase, outs):
                                base = 1 + (r + 1) * WP + dt0
                                r0 = ap_t[:, base:base + NB]
                                if t < 8:
                                    rhs = bass_mod.AP(
                                        tensor=r0.tensor, offset=r0.offset,
                                        ap=[r0.ap[0], [dpair, 2], r0.ap[1]])
                                    nc.tensor.matmul(po, lhs, rhs,
                                                     start=False,
                                                     stop=False,
                                                     perf_mode=DoubleRow)
                                else:
                                    nc.tensor.matmul(po, lhs, r0,
                                                     start=False, stop=True)
                        pv = pt4[:, :, 0:NB]
                        ut = u_pool.tile([128, 4, NB], f32, name="ut",
                                         tag="ut")
                        nc.scalar.activation(out=ut, in_=pv, func=Act.Abs,
                                             bias=bA_ap, scale=sA_ap)
                        vt = v_pool.tile([128, 4, NB], mybir.dt.bfloat16,
                                         name="vt", tag="vt")
                        # out = c1*(s+k) + |sA*(s+k) + bA| in one DVE op,
                        # rounded to bf16 on the way out
                        nc.vector.scalar_tensor_tensor(
                            out=vt, in0=pv, scalar=c1_ap, in1=ut,
                            op0=Alu.mult, op1=Alu.add)
                        # output DMA on the Activation HWDGE: cross-engine
                        # queue mixing costs bandwidth when both streams are
                        # saturated, but at the real pacing it measures
                        # faster than sharing the SP queue, where a result
                        # that isn't ready yet blocks queued input DMAs
                        y0 = BAND * b + 8 * half
                        nc.scalar.dma_start(
                            out=y_out[h * 128:(h + 1) * 128,
                                      y0 * WP:(y0 + 8) * WP],
                            in_=vt.rearrange("p f n -> p (f n)"))

                # half-band granularity: the first half of band b only
                # needs pooled rows up to 16b+8 (chunk 2b+1), the second
                # half needs chunk 2b+2's halo row
                for c in range(N_CHUNKS):
                    emit_chunk(c)
                    if c % 2 == 1:
                        emit_conv(c // 2, 0)
                    elif c >= 2:
                        emit_conv(c // 2 - 1, 1)
                        apad.pop(c // 2 - 1)
                emit_conv(N_BANDS - 1, 1)
                apad.pop(N_BANDS - 1)
    nc.compile()
    return nc


def get_program(repeats: int = 1):
    if repeats not in _PROGRAMS:
        _PROGRAMS[repeats] = _build_program(repeats)
    return _PROGRAMS[repeats]


def host_prep(weight, move0_bias, pr_bias0, prelu_alpha, pr_bias1):
    import ml_dtypes

    w = np.asarray(weight, dtype=np.float32)  # [COUT, CIN, 3, 3]
    sw = np.sign(w).astype(np.float32)
    # lhsT layout [ci, tap, co]
    wt = np.ascontiguousarray(
        np.transpose(sw, (1, 2, 3, 0)).reshape(CIN, 9, COUT)
    ).astype(ml_dtypes.float8_e4m3)

    scale = np.mean(np.abs(w), axis=(1, 2, 3), dtype=np.float32)  # [COUT]
    al = np.asarray(prelu_alpha, dtype=np.float32).reshape(COUT)
    b0 = np.asarray(pr_bias0, dtype=np.float32).reshape(COUT)
    b1 = np.asarray(pr_bias1, dtype=np.float32).reshape(COUT)
    c1 = 0.5 * (1.0 + al) * scale
    c2 = 0.5 * (1.0 + al) * b0 + b1
    c3 = 0.5 * (1.0 - al)
    sA = c3 * scale
    bA = c3 * b0

    # bias tap: 128 lhsT rows of bf16(k/128) summed by a ones matmul.
    # Compensate the Abs bias with the exact summed value so only the tiny
    # c1*(k - k_eff) residual remains.
    kq = (c2 / c1 / 128.0).astype(ml_dtypes.bfloat16)
    k_eff = 128.0 * kq.astype(np.float32)
    bA = bA - sA * k_eff
    kb = np.broadcast_to(kq.reshape(1, COUT), (128, COUT)).copy()

    ct = np.zeros((128, 9), dtype=np.float32)
    ct[:, 0] = np.asarray(move0_bias, dtype=np.float32).reshape(CIN)
    for h in (0, 1):
        sl = slice(h * 128, (h + 1) * 128)
        ct[:, 1 + 4 * h] = c1[sl]
        ct[:, 2 + 4 * h] = c2[sl]
        ct[:, 3 + 4 * h] = sA[sl]
        ct[:, 4 + 4 * h] = bA[sl]
    return wt, ct, kb


def kernel(x, weight, move0_bias, pr_bias0, prelu_alpha, pr_bias1):
    from concourse.bass_utils import run_bass_kernel_spmd

    x = np.asarray(x, dtype=np.float32)
    wt, ct, kb = host_prep(weight, move0_bias, pr_bias0, prelu_alpha,
                           pr_bias1)
    nc = get_program()
    in_maps = [{"x": x[c], "wt": wt, "ct": ct, "kb": kb}
               for c in range(N_CORES)]
    res = run_bass_kernel_spmd(nc, in_maps, list(range(N_CORES)))
    WPAD = W2 + 2
    y = np.stack([np.asarray(res.results[c]["y"]).reshape(COUT, H2, WPAD)
                  for c in range(N_CORES)], axis=0)
    return np.ascontiguousarray(y[:, :, :, 1:W2 + 1].astype(np.float32))

